# revision 58
# baseline (speedup 1.0000x reference)
"""Longformer sliding-window self-attention on 8 Trainium2 NeuronCores.

Problem: B=2, S=4096, E=768, H=12 heads, D=64, one-sided window W=256.
Sharding: batch*head parallel - core i handles batch i//4, heads 3*(i%4)..+3.
Each core is fully independent (no collectives).

v2 design (vs v1): bf16 matmul operands (half-cost LDWEIGHTS, half DMA),
3 projection groups [k0|k1],[q0|q1],[k2|q2] using explicit tile_position
for head2 (PE row 64 weights vs SBUF base-0 lhsT), additive band masks
preloaded into PSUM by GpSimd so QK matmuls accumulate on top of them
(start=False) and the mask-add leaves the critical QK->exp chain, exps
batched per psum tile ([128,1024] + [128,512] per task), bv folded into
the V projection (exact: sum(probs)=1), and software pipelining of tasks
i=(block m, head h): emit QK(i), PV(i-1), tail(i-2) so the PE always has
ready work and sustains the 2.4 GHz p-state.

Per-core device program (SPMD, identical on all 8 cores):
  inputs (host-prepared, bf16 unless noted):
    hT6   [6, 128, 4096]  hidden[b].T tiled over E (contraction tiles)
    wqk3  [128, 1152]     3 proj groups x 6 ki x 128 cols, q pre-scaled
    wv    [128, 1152]     6 ki x 192 cols (Wv for the 3 heads)
    masks [128, 1024]     f32: [ma|mb] and [mc|md] combined pairs
    identB[128, 128]      identity for PE transpose
    bqk   [128, 3]        f32 per-group per-partition bias
    bvb   [128, 192]      f32 bv broadcast (folded into V projection)
    ones  [128, 96]       ones column for the softmax denominator
  output:
    out   [3, 4096, 64]   f32 per-head attention output

  Phase B (projections): kT/qT in transposed [d, S] layout via
  lhsT=wqk3-group rhs=hT-chunk matmuls (ap=512); v in natural [S, d]
  layout via lhsT=hT-subtile rhs=wv (ap=192), psum->SBUF add of bvb on
  GpSimd. kq group psum -> SBUF copy with per-partition bias on ACT.

  Phase C (attention): task i = (m, h). Scores^T [key, query] for the
  6 (edge: 4) key tiles land in one [128,1024] psum (4 quarters) plus
  one [128,512] psum (2 halves), on top of preloaded masks. One or two
  Exp ACT ops -> et (bf16). PV: 6 accumulating matmuls lhsT=v[128,65]
  (ones col 64 -> denominator row), out [65,256] psum half (parity).
  Tail: DVE copy -> ot bf16, 2 PE transposes -> [128,65] bf16 psum,
  reciprocal of col 64, multiply, one DMA per task ([256,64] f32).
"""

import numpy as np
from ml_dtypes import bfloat16

import concourse.bass as bass
import concourse.bacc as bacc
import concourse.mybir as mybir
import concourse.tile as tile
from concourse.bass_utils import run_bass_kernel_spmd

B, S, E, H, D, W = 2, 4096, 768, 12, 64, 256
NCORES = 8
HPC = 3  # heads per core
QB = 256  # queries per attention block
NBLK = S // QB  # 16
KI = E // 128  # 6 contraction tiles
PCH = 512  # projection N-chunk (along S)
NCH = S // PCH  # 8
F32 = mybir.dt.float32
BF16 = mybir.dt.bfloat16
NEG = -1e30
Act = mybir.ActivationFunctionType
Alu = mybir.AluOpType
NTASK = NBLK * HPC  # 48


def _mask_np():
    """Fine 0/1 masks [2][128][128]: lo (valid iff t' >= p), hi (t' <= p).

    t' = key offset within a 128 key tile (partition dim), p = query
    offset within a 128-query tile (free dim). Applied multiplicatively
    to the exp'd probs (et) on DVE: the extreme key tiles (kappa = t-2
    and t+2) of each query tile t are triangular; the middle 3 are fully
    valid."""
    p = np.arange(128)[None, :]
    t = np.arange(128)[:, None]
    lo = np.where(t >= p, 1.0, 0.0)
    hi = np.where(t <= p, 1.0, 0.0)
    return np.stack([lo, hi]).astype(np.float32)


def _qtile_plan(t):
    """(lo, hi) inclusive key-tile range for 128-query tile t; the slot of
    key tile kappa within the 5-slot window is kappa - (t-2)."""
    return max(0, t - 2), min(S // 128 - 1, t + 2)


def _build_nc():
    nc = bacc.Bacc()
    # hT6 layout [p][chunk][ki][s]: per-partition contiguous 6KB per chunk
    ht_d = nc.declare_dram_parameter("hT6", [128, NCH, KI, PCH], BF16, isOutput=False)
    wqk_d = nc.declare_dram_parameter("wqk3", [128, HPC * KI * 128], BF16, isOutput=False)
    wv_d = nc.declare_dram_parameter("wv", [128, KI * 192], BF16, isOutput=False)
    msk_d = nc.declare_dram_parameter("masks", [128, 2 * 128], BF16, isOutput=False)
    idn_d = nc.declare_dram_parameter("identB", [128, 128], BF16, isOutput=False)
    bqk_d = nc.declare_dram_parameter("bqk", [128, HPC], F32, isOutput=False)
    bvb_d = nc.declare_dram_parameter("bvb", [128, 192], F32, isOutput=False)
    out_d = nc.declare_dram_parameter("out", [HPC, S, D], F32, isOutput=True)

    with tile.TileContext(nc) as tc:
        with (
            tc.tile_pool(name="const", bufs=1) as const,
            tc.tile_pool(name="hpool", bufs=2) as hpool,
            tc.tile_pool(name="work", bufs=2) as work,
            tc.tile_pool(name="outp", bufs=4) as outp,
            tc.tile_pool(name="ps", bufs=1, space="PSUM") as psp,
        ):
            # ---- persistent tiles (DMA order: critical-path first) ----
            wqk = const.tile([128, HPC, KI, 128], BF16)
            nc.sync.dma_start(wqk[:, 0], wqk_d[:, 0 : KI * 128])
            hts0 = hpool.tile([128, KI, PCH], BF16, tag="ht")
            nc.sync.dma_start(hts0, ht_d[:, 0])
            nc.sync.dma_start(wqk[:, 1:3], wqk_d[:, KI * 128 :])
            bqk = const.tile([128, HPC], F32)
            nc.sync.dma_start(bqk, bqk_d[:])
            wv = const.tile([128, KI, 192], BF16)
            nc.sync.dma_start(wv, wv_d[:])
            bvb = const.tile([128, 192], F32)
            nc.sync.dma_start(bvb, bvb_d[:])
            masks = const.tile([128, 2, 128], BF16)
            identB = const.tile([128, 128], BF16)

            # transposed [d, S] projections: G0=[k0|k1], G1=[q0|q1], G2=[k2|q2].
            # Matmul lhsT/rhs must share the SB partition base, so q2 is
            # DMA-copied (per chunk) from partitions 64:128 down to base 0.
            kq = const.tile([128, HPC, S], BF16)
            q2b = const.tile([128, S], BF16)

            def kT_ap(h, sl):
                return (kq[0:64, 0, sl], kq[64:128, 0, sl], kq[0:64, 2, sl])[h]

            def qT_ap(h, sl):
                return (kq[0:64, 1, sl], kq[64:128, 1, sl], q2b[0:64, sl])[h]

            # v in natural [S, d] layout: [s-tile-of-128, head, key-tile, d+ones].
            # memset to 1.0; the V-projection TTs overwrite cols 0:64, leaving
            # col 64 = 1.0 (softmax denominator ones column).
            v_sb = const.tile([128, HPC, S // 128, D + 1], BF16)
            nc.vector.memset(v_sb, 1.0)

            # shared psum tiles with parity-aliased regions
            ps_o = psp.tile([128, 512], F32, tag="po")  # PV out, 2 gens x [65,256]
            ps_t = psp.tile([128, 2, 2, 66], BF16, tag="pt")  # transposes, parity x j

            # ---- Phase B emitters: projection pieces (for fine interleave) ----
            def emit_ht_dma(c):
                if c == 0:
                    return hts0
                hts = hpool.tile([128, KI, PCH], BF16, tag="ht", name="hts")
                nc.sync.dma_start(hts, ht_d[:, c])
                return hts

            def emit_proj_group(c, g, hts):
                psq = psp.tile([128, PCH], F32, tag="sA", bufs=2, name="psq")
                for ki in range(KI):
                    nc.tensor.matmul(
                        psq,
                        wqk[:, g, ki, :],
                        hts[:, ki, :],
                        start=(ki == 0),
                        stop=(ki == KI - 1),
                    )
                nc.scalar.activation(
                    kq[:, g, c * PCH : (c + 1) * PCH],
                    psq,
                    Act.Identity,
                    bias=bqk[:, g : g + 1],
                    scale=1.0,
                )
                if g == 2:
                    nc.sync.dma_start(
                        q2b[0:64, c * PCH : (c + 1) * PCH],
                        kq[64:128, 2, c * PCH : (c + 1) * PCH],
                    )
                if c == 0 and g == 2:
                    # needed only by attention; issued behind the hot-path DMAs
                    nc.sync.dma_start(masks, msk_d[:])
                    nc.sync.dma_start(identB, idn_d[:])

            def emit_v_j(c, j, hts, psv):
                for ki in range(KI):
                    nc.tensor.matmul(
                        psv[:, j, 0:192],
                        hts[:, ki, j * 128 : (j + 1) * 128],
                        wv[:, ki, :],
                        start=(ki == 0),
                        stop=(ki == KI - 1),
                    )
                g = (PCH // 128) * c + j
                nc.vector.tensor_tensor(
                    v_sb[:, :, g, 0:D],
                    psv[:, j, 0:192].rearrange("p (h d) -> p h d", h=HPC),
                    bvb.rearrange("p (h d) -> p h d", h=HPC),
                    Alu.add,
                )

            def emit_chunk_pieces(c):
                hts = emit_ht_dma(c)
                pieces = [lambda g=g: emit_proj_group(c, g, hts) for g in range(HPC)]
                psv = [None]

                def vj(j):
                    if psv[0] is None:
                        psv[0] = psp.tile(
                            [128, PCH // 128, 256], F32, tag="sA", bufs=2, name="psv"
                        )
                    emit_v_j(c, j, hts, psv[0])

                pieces += [lambda j=j: vj(j) for j in range(PCH // 128)]
                return pieces

            # ---- Phase C: attention, software-pipelined tasks ----
            pend = {}

            def emit_qk(i):
                # task i = (block m, head h) covers the two 128-query tiles
                # t = 2m, 2m+1, each with a 5-slot key window (kappa = t-2 ..
                # t+2 clipped); slots live at et/psum cols qh*640 + slot*128.
                # Unused edge slots hold exp(garbage) but are never read.
                m, h = divmod(i, HPC)
                ps_s = psp.tile([128, 1280], F32, tag="sA", bufs=2, name="ps_s")
                for qh in range(2):
                    t = 2 * m + qh
                    lo, hi = _qtile_plan(t)
                    qsl = slice(t * 128, (t + 1) * 128)
                    for ka in range(lo, hi + 1):
                        o = ka - (t - 2)
                        col = qh * 640 + o * 128
                        nc.tensor.matmul(
                            ps_s[:, col : col + 128],
                            kT_ap(h, slice(ka * 128, (ka + 1) * 128)),
                            qT_ap(h, qsl),
                            start=True,
                            stop=True,
                        )
                et = work.tile([128, 1280], BF16, tag="et", name="et", bufs=4)
                nc.scalar.activation(et, ps_s, Act.Exp)
                for qh in range(2):
                    t = 2 * m + qh
                    has_lo, has_hi = t >= 2, t <= S // 128 - 3
                    base = qh * 640
                    if has_lo and has_hi:
                        # one strided TT covering slots 0 (lo) and 4 (hi)
                        nc.vector.tensor_tensor(
                            et[:, base : base + 640].rearrange(
                                "p (s c) -> p s c", c=128
                            )[:, 0:5:4, :],
                            et[:, base : base + 640].rearrange(
                                "p (s c) -> p s c", c=128
                            )[:, 0:5:4, :],
                            masks,
                            Alu.mult,
                        )
                    elif has_hi:
                        nc.vector.tensor_tensor(
                            et[:, base + 512 : base + 640],
                            et[:, base + 512 : base + 640],
                            masks[:, 1, :],
                            Alu.mult,
                        )
                    elif has_lo:
                        nc.vector.tensor_tensor(
                            et[:, base : base + 128],
                            et[:, base : base + 128],
                            masks[:, 0, :],
                            Alu.mult,
                        )
                pend[i] = (m, h, et)

            def emit_pv(i):
                m, h, et = pend[i]
                for qh in range(2):
                    t = 2 * m + qh
                    lo, hi = _qtile_plan(t)
                    po = ps_o[
                        0 : D + 1,
                        (i % 2) * 256 + qh * 128 : (i % 2) * 256 + qh * 128 + 128,
                    ]
                    for ka in range(lo, hi + 1):
                        o = ka - (t - 2)
                        col = qh * 640 + o * 128
                        nc.tensor.matmul(
                            po,
                            v_sb[:, h, ka, :],
                            et[:, col : col + 128],
                            start=(ka == lo),
                            stop=(ka == hi),
                        )

            def emit_tail(i):
                m, h, et = pend.pop(i)
                par = i % 2
                ot = outp.tile([128, 256], BF16, tag="ot", name="ot")
                nc.vector.tensor_copy(
                    ot[0 : D + 1, :], ps_o[0 : D + 1, par * 256 : par * 256 + 256]
                )
                for j in range(2):
                    nc.tensor.transpose(
                        ps_t[:, par, j, 0 : D + 1],
                        ot[0 : D + 1, j * 128 : (j + 1) * 128],
                        identB[0 : D + 1, 0 : D + 1],
                    )
                rc = outp.tile([128, 2], F32, tag="rc", name="rc")
                nc.vector.reciprocal(rc, ps_t[:, par, :, D : D + 1])
                ob = outp.tile([128, 2, D], F32, tag="ob", name="ob")
                for j in range(2):
                    nc.vector.tensor_scalar(
                        ob[:, j, :], ps_t[:, par, j, 0:D], rc[:, j : j + 1], None,
                        Alu.mult,
                    )
                nc.sync.dma_start(
                    out_d.rearrange("h (m two p) d -> h m p two d", two=2, p=128)[
                        h, m
                    ],
                    ob,
                )

            # ---- unified emission: interleave projection pieces with the
            # attention tasks they unblock (block m needs chunks <= (m+1)//2),
            # one piece between consecutive tasks so ACT/DVE attention work
            # spreads across the whole run and only the PE paces. Attention
            # tasks stay software-pipelined: QK(i), PV(i-1), tail(i-2).
            i = 0

            credit = [0]

            def pump(n, flush=False):
                # Process tasks in PAIRS with same-kind ops adjacent (2 tails,
                # 2 QKs, 2 PVs): the PE pays a half-speed matmul after each
                # weight-geometry switch (transpose / K=64 QK / K=128 PV), so
                # batching halves the switch count. Credits accumulate across
                # calls so pairs actually form. PV lags QK by 2 tasks so et
                # (exp+mask) is always ready; tails lag by 4.
                nonlocal i
                credit[0] += n
                while credit[0] >= 2 or (flush and credit[0] > 0):
                    k = min(2, credit[0])
                    for d in range(k):
                        if 0 <= i + d - 4 < NTASK:
                            emit_tail(i + d - 4)
                    for d in range(k):
                        if i + d < NTASK:
                            emit_qk(i + d)
                    for d in range(k):
                        if 0 <= i + d - 2 < NTASK:
                            emit_pv(i + d - 2)
                    i += k
                    credit[0] -= k

            # tasks of block m are emitted only after every piece of chunk
            # K(m) = min((m+1)//2, NCH-1): reads must follow their producing
            # writes in program order for Tile to insert the dependency.
            for c in range(NCH):
                pieces = emit_chunk_pieces(c)
                avail = HPC * len(
                    [m for m in range(NBLK) if min((m + 1) // 2, NCH - 1) < c]
                )
                base = i
                for pi, piece in enumerate(pieces):
                    piece()
                    budget = avail - base
                    share = budget * (pi + 1) // len(pieces) - budget * pi // len(
                        pieces
                    )
                    pump(min(share, avail - i - credit[0]))
            pump(NTASK + 4 - i - credit[0], flush=True)
    nc.compile()
    return nc


_CACHE = {}


def _get_nc():
    if "nc" not in _CACHE:
        _CACHE["nc"] = _build_nc()
    return _CACHE["nc"]


def make_in_maps(hidden_states, Wq, bq, Wk, bk, Wv, bv):
    hidden_states = np.asarray(hidden_states, dtype=np.float32)
    Wq = np.asarray(Wq, dtype=np.float32)
    Wk = np.asarray(Wk, dtype=np.float32)
    Wv = np.asarray(Wv, dtype=np.float32)
    bq = np.asarray(bq, dtype=np.float32)
    bk = np.asarray(bk, dtype=np.float32)
    bv = np.asarray(bv, dtype=np.float32)
    scale = 1.0 / float(np.sqrt(D))
    # masks layout [p][kind][128]: masks_np[kind][p][c] -> [p][kind][c]
    masks = np.ascontiguousarray(
        _mask_np().transpose(1, 0, 2).reshape(128, 256)
    ).astype(bfloat16)
    identB = np.eye(128, dtype=np.float32).astype(bfloat16)
    in_maps = []
    for core in range(NCORES):
        b = core // (NCORES // B)
        h0 = HPC * (core % (NCORES // B))
        hsl = [slice(D * (h0 + hh), D * (h0 + hh + 1)) for hh in range(HPC)]
        cols = slice(D * h0, D * (h0 + HPC))
        G = np.empty((HPC, E, 128), np.float32)
        G[0][:, 0:D] = Wk[:, hsl[0]]
        G[0][:, D:128] = Wk[:, hsl[1]]
        G[1][:, 0:D] = Wq[:, hsl[0]] * scale
        G[1][:, D:128] = Wq[:, hsl[1]] * scale
        G[2][:, 0:D] = Wk[:, hsl[2]]
        G[2][:, D:128] = Wq[:, hsl[2]] * scale
        # wqk3[p, g, ki, m] = G[g][ki*128+p, m]
        wqk3 = G.reshape(HPC, KI, 128, 128).transpose(2, 0, 1, 3)
        bqk = np.empty((128, HPC), np.float32)
        bqk[0:D, 0] = bk[hsl[0]]
        bqk[D:128, 0] = bk[hsl[1]]
        bqk[0:D, 1] = bq[hsl[0]] * scale
        bqk[D:128, 1] = bq[hsl[1]] * scale
        bqk[0:D, 2] = bk[hsl[2]]
        bqk[D:128, 2] = bq[hsl[2]] * scale
        # wv[p, ki, n] = Wv[ki*128+p, cols[n]]
        wv_p = Wv[:, cols].reshape(KI, 128, HPC * D).transpose(1, 0, 2)
        bvb = np.broadcast_to(bv[cols], (128, HPC * D)).copy()
        # hT6[p, c, ki, s] = hidden[b].T[ki*128+p, c*512+s]
        hT6 = np.ascontiguousarray(
            hidden_states[b].T.reshape(KI, 128, NCH, PCH).transpose(1, 2, 0, 3)
        ).astype(bfloat16)
        in_maps.append(
            dict(
                hT6=hT6,
                wqk3=np.ascontiguousarray(wqk3).reshape(128, HPC * KI * 128).astype(bfloat16),
                wv=np.ascontiguousarray(wv_p).reshape(128, KI * 192).astype(bfloat16),
                masks=np.ascontiguousarray(masks),
                identB=identB,
                bqk=bqk,
                bvb=bvb,
            )
        )
    return in_maps


def kernel(hidden_states, Wq, bq, Wk, bk, Wv, bv):
    in_maps = make_in_maps(hidden_states, Wq, bq, Wk, bk, Wv, bv)
    res = run_bass_kernel_spmd(_get_nc(), in_maps, list(range(NCORES)))
    kernel.last = res
    out = np.empty((B, S, E), np.float32)
    for core in range(NCORES):
        r = res.results[core]["out"]
        b = core // (NCORES // B)
        h0 = HPC * (core % (NCORES // B))
        for hh in range(HPC):
            out[b, :, D * (h0 + hh) : D * (h0 + hh + 1)] = r[hh]
    return out


# revision 60
# speedup vs baseline: 1.0047x; 1.0047x over previous
"""Longformer sliding-window self-attention on 8 Trainium2 NeuronCores.

Problem: B=2, S=4096, E=768, H=12 heads, D=64, one-sided window W=256.
Sharding: batch*head parallel - core i handles batch i//4, heads 3*(i%4)..+3.
Each core is fully independent (no collectives).

v2 design (vs v1): bf16 matmul operands (half-cost LDWEIGHTS, half DMA),
3 projection groups [k0|k1],[q0|q1],[k2|q2] using explicit tile_position
for head2 (PE row 64 weights vs SBUF base-0 lhsT), additive band masks
preloaded into PSUM by GpSimd so QK matmuls accumulate on top of them
(start=False) and the mask-add leaves the critical QK->exp chain, exps
batched per psum tile ([128,1024] + [128,512] per task), bv folded into
the V projection (exact: sum(probs)=1), and software pipelining of tasks
i=(block m, head h): emit QK(i), PV(i-1), tail(i-2) so the PE always has
ready work and sustains the 2.4 GHz p-state.

Per-core device program (SPMD, identical on all 8 cores):
  inputs (host-prepared, bf16 unless noted):
    hT6   [6, 128, 4096]  hidden[b].T tiled over E (contraction tiles)
    wqk3  [128, 1152]     3 proj groups x 6 ki x 128 cols, q pre-scaled
    wv    [128, 1152]     6 ki x 192 cols (Wv for the 3 heads)
    masks [128, 1024]     f32: [ma|mb] and [mc|md] combined pairs
    identB[128, 128]      identity for PE transpose
    bqk   [128, 3]        f32 per-group per-partition bias
    bvb   [128, 192]      f32 bv broadcast (folded into V projection)
    ones  [128, 96]       ones column for the softmax denominator
  output:
    out   [3, 4096, 64]   f32 per-head attention output

  Phase B (projections): kT/qT in transposed [d, S] layout via
  lhsT=wqk3-group rhs=hT-chunk matmuls (ap=512); v in natural [S, d]
  layout via lhsT=hT-subtile rhs=wv (ap=192), psum->SBUF add of bvb on
  GpSimd. kq group psum -> SBUF copy with per-partition bias on ACT.

  Phase C (attention): task i = (m, h). Scores^T [key, query] for the
  6 (edge: 4) key tiles land in one [128,1024] psum (4 quarters) plus
  one [128,512] psum (2 halves), on top of preloaded masks. One or two
  Exp ACT ops -> et (bf16). PV: 6 accumulating matmuls lhsT=v[128,65]
  (ones col 64 -> denominator row), out [65,256] psum half (parity).
  Tail: DVE copy -> ot bf16, 2 PE transposes -> [128,65] bf16 psum,
  reciprocal of col 64, multiply, one DMA per task ([256,64] f32).
"""

import numpy as np
from ml_dtypes import bfloat16

import concourse.bass as bass
import concourse.bacc as bacc
import concourse.mybir as mybir
import concourse.tile as tile
from concourse.bass_utils import run_bass_kernel_spmd

B, S, E, H, D, W = 2, 4096, 768, 12, 64, 256
NCORES = 8
HPC = 3  # heads per core
QB = 256  # queries per attention block
NBLK = S // QB  # 16
KI = E // 128  # 6 contraction tiles
PCH = 512  # projection N-chunk (along S)
NCH = S // PCH  # 8
F32 = mybir.dt.float32
BF16 = mybir.dt.bfloat16
NEG = -1e30
Act = mybir.ActivationFunctionType
Alu = mybir.AluOpType
NTASK = NBLK * HPC  # 48


def _mask_np():
    """Fine 0/1 masks [2][128][128]: lo (valid iff t' >= p), hi (t' <= p).

    t' = key offset within a 128 key tile (partition dim), p = query
    offset within a 128-query tile (free dim). Applied multiplicatively
    to the exp'd probs (et) on DVE: the extreme key tiles (kappa = t-2
    and t+2) of each query tile t are triangular; the middle 3 are fully
    valid."""
    p = np.arange(128)[None, :]
    t = np.arange(128)[:, None]
    lo = np.where(t >= p, 1.0, 0.0)
    hi = np.where(t <= p, 1.0, 0.0)
    return np.stack([lo, hi]).astype(np.float32)


def _qtile_plan(t):
    """(lo, hi) inclusive key-tile range for 128-query tile t; the slot of
    key tile kappa within the 5-slot window is kappa - (t-2)."""
    return max(0, t - 2), min(S // 128 - 1, t + 2)


def _build_nc():
    nc = bacc.Bacc()
    # hT6 layout [p][chunk][ki][s]: per-partition contiguous 6KB per chunk
    ht_d = nc.declare_dram_parameter("hT6", [128, NCH, KI, PCH], BF16, isOutput=False)
    wqk_d = nc.declare_dram_parameter("wqk3", [128, HPC * KI * 128], BF16, isOutput=False)
    wv_d = nc.declare_dram_parameter("wv", [128, KI * 192], BF16, isOutput=False)
    msk_d = nc.declare_dram_parameter("masks", [128, 2 * 128], BF16, isOutput=False)
    idn_d = nc.declare_dram_parameter("identB", [128, 128], BF16, isOutput=False)
    bqk_d = nc.declare_dram_parameter("bqk", [128, HPC], F32, isOutput=False)
    bvb_d = nc.declare_dram_parameter("bvb", [128, 192], F32, isOutput=False)
    out_d = nc.declare_dram_parameter("out", [HPC, S, D], F32, isOutput=True)

    with tile.TileContext(nc) as tc:
        with (
            tc.tile_pool(name="const", bufs=1) as const,
            tc.tile_pool(name="hpool", bufs=3) as hpool,
            tc.tile_pool(name="work", bufs=2) as work,
            tc.tile_pool(name="outp", bufs=4) as outp,
            tc.tile_pool(name="ps", bufs=1, space="PSUM") as psp,
        ):
            # ---- persistent tiles (DMA order: critical-path first) ----
            wqk = const.tile([128, HPC, KI, 128], BF16)
            nc.sync.dma_start(wqk[:, 0], wqk_d[:, 0 : KI * 128])
            hts0 = hpool.tile([128, KI, PCH], BF16, tag="ht")
            nc.sync.dma_start(hts0, ht_d[:, 0])
            bqk = const.tile([128, HPC], F32)
            nc.sync.dma_start(bqk, bqk_d[:])
            nc.sync.dma_start(wqk[:, 1:3], wqk_d[:, KI * 128 :])
            wv = const.tile([128, KI, 192], BF16)
            nc.sync.dma_start(wv, wv_d[:])
            bvb = const.tile([128, 192], F32)
            nc.sync.dma_start(bvb, bvb_d[:])
            masks = const.tile([128, 2, 128], BF16)
            identB = const.tile([128, 128], BF16)

            # transposed [d, S] projections: G0=[k0|k1], G1=[q0|q1], G2=[k2|q2].
            # Matmul lhsT/rhs must share the SB partition base, so q2 is
            # DMA-copied (per chunk) from partitions 64:128 down to base 0.
            kq = const.tile([128, HPC, S], BF16)
            q2b = const.tile([128, S], BF16)

            def kT_ap(h, sl):
                return (kq[0:64, 0, sl], kq[64:128, 0, sl], kq[0:64, 2, sl])[h]

            def qT_ap(h, sl):
                return (kq[0:64, 1, sl], kq[64:128, 1, sl], q2b[0:64, sl])[h]

            # v in natural [S, d] layout: [s-tile-of-128, head, key-tile, d+ones].
            # memset to 1.0; the V-projection TTs overwrite cols 0:64, leaving
            # col 64 = 1.0 (softmax denominator ones column).
            v_sb = const.tile([128, HPC, S // 128, D + 1], BF16)
            nc.vector.memset(v_sb, 1.0)

            # shared psum tiles with parity-aliased regions
            ps_o = psp.tile([128, 512], F32, tag="po")  # PV out, 2 gens x [65,256]
            ps_t = psp.tile([128, 2, 2, 66], BF16, tag="pt")  # transposes, parity x j

            # ---- Phase B emitters: projection pieces (for fine interleave) ----
            def emit_ht_dma(c):
                if c == 0:
                    return hts0
                hts = hpool.tile([128, KI, PCH], BF16, tag="ht", name="hts")
                nc.sync.dma_start(hts, ht_d[:, c])
                return hts

            def emit_proj_group(c, g, hts):
                psq = psp.tile([128, PCH], F32, tag="sA", bufs=2, name="psq")
                for ki in range(KI):
                    nc.tensor.matmul(
                        psq,
                        wqk[:, g, ki, :],
                        hts[:, ki, :],
                        start=(ki == 0),
                        stop=(ki == KI - 1),
                    )
                nc.scalar.activation(
                    kq[:, g, c * PCH : (c + 1) * PCH],
                    psq,
                    Act.Identity,
                    bias=bqk[:, g : g + 1],
                    scale=1.0,
                )
                if g == 2:
                    nc.sync.dma_start(
                        q2b[0:64, c * PCH : (c + 1) * PCH],
                        kq[64:128, 2, c * PCH : (c + 1) * PCH],
                    )
                if c == 0 and g == 2:
                    # needed only by attention; issued behind the hot-path DMAs
                    nc.sync.dma_start(masks, msk_d[:])
                    nc.sync.dma_start(identB, idn_d[:])

            def emit_v_j(c, j, hts, psv):
                for ki in range(KI):
                    nc.tensor.matmul(
                        psv[:, j, 0:192],
                        hts[:, ki, j * 128 : (j + 1) * 128],
                        wv[:, ki, :],
                        start=(ki == 0),
                        stop=(ki == KI - 1),
                    )
                g = (PCH // 128) * c + j
                nc.vector.tensor_tensor(
                    v_sb[:, :, g, 0:D],
                    psv[:, j, 0:192].rearrange("p (h d) -> p h d", h=HPC),
                    bvb.rearrange("p (h d) -> p h d", h=HPC),
                    Alu.add,
                )

            def emit_chunk_pieces(c):
                hts = emit_ht_dma(c)
                pieces = [lambda g=g: emit_proj_group(c, g, hts) for g in range(HPC)]
                psv = [None]

                def vj(j):
                    if psv[0] is None:
                        psv[0] = psp.tile(
                            [128, PCH // 128, 256], F32, tag="sA", bufs=2, name="psv"
                        )
                    emit_v_j(c, j, hts, psv[0])

                pieces += [lambda j=j: vj(j) for j in range(PCH // 128)]
                return pieces

            # ---- Phase C: attention, software-pipelined tasks ----
            pend = {}

            def emit_qk(i):
                # task i = (block m, head h) covers the two 128-query tiles
                # t = 2m, 2m+1, each with a 5-slot key window (kappa = t-2 ..
                # t+2 clipped); slots live at et/psum cols qh*640 + slot*128.
                # Unused edge slots hold exp(garbage) but are never read.
                m, h = divmod(i, HPC)
                ps_s = psp.tile([128, 1280], F32, tag="sA", bufs=2, name="ps_s")
                for qh in range(2):
                    t = 2 * m + qh
                    lo, hi = _qtile_plan(t)
                    qsl = slice(t * 128, (t + 1) * 128)
                    for ka in range(lo, hi + 1):
                        o = ka - (t - 2)
                        col = qh * 640 + o * 128
                        nc.tensor.matmul(
                            ps_s[:, col : col + 128],
                            kT_ap(h, slice(ka * 128, (ka + 1) * 128)),
                            qT_ap(h, qsl),
                            start=True,
                            stop=True,
                        )
                et = work.tile([128, 1280], BF16, tag="et", name="et", bufs=4)
                nc.scalar.activation(et, ps_s, Act.Exp)
                for qh in range(2):
                    t = 2 * m + qh
                    has_lo, has_hi = t >= 2, t <= S // 128 - 3
                    base = qh * 640
                    if has_lo and has_hi:
                        # one strided TT covering slots 0 (lo) and 4 (hi)
                        nc.vector.tensor_tensor(
                            et[:, base : base + 640].rearrange(
                                "p (s c) -> p s c", c=128
                            )[:, 0:5:4, :],
                            et[:, base : base + 640].rearrange(
                                "p (s c) -> p s c", c=128
                            )[:, 0:5:4, :],
                            masks,
                            Alu.mult,
                        )
                    elif has_hi:
                        nc.vector.tensor_tensor(
                            et[:, base + 512 : base + 640],
                            et[:, base + 512 : base + 640],
                            masks[:, 1, :],
                            Alu.mult,
                        )
                    elif has_lo:
                        nc.vector.tensor_tensor(
                            et[:, base : base + 128],
                            et[:, base : base + 128],
                            masks[:, 0, :],
                            Alu.mult,
                        )
                pend[i] = (m, h, et)

            def emit_pv(i):
                m, h, et = pend[i]
                for qh in range(2):
                    t = 2 * m + qh
                    lo, hi = _qtile_plan(t)
                    po = ps_o[
                        0 : D + 1,
                        (i % 2) * 256 + qh * 128 : (i % 2) * 256 + qh * 128 + 128,
                    ]
                    for ka in range(lo, hi + 1):
                        o = ka - (t - 2)
                        col = qh * 640 + o * 128
                        nc.tensor.matmul(
                            po,
                            v_sb[:, h, ka, :],
                            et[:, col : col + 128],
                            start=(ka == lo),
                            stop=(ka == hi),
                        )

            def emit_tail(i):
                m, h, et = pend.pop(i)
                par = i % 2
                ot = outp.tile([128, 256], BF16, tag="ot", name="ot")
                nc.vector.tensor_copy(
                    ot[0 : D + 1, :], ps_o[0 : D + 1, par * 256 : par * 256 + 256]
                )
                for j in range(2):
                    nc.tensor.transpose(
                        ps_t[:, par, j, 0 : D + 1],
                        ot[0 : D + 1, j * 128 : (j + 1) * 128],
                        identB[0 : D + 1, 0 : D + 1],
                    )
                rc = outp.tile([128, 2], F32, tag="rc", name="rc")
                nc.vector.reciprocal(rc, ps_t[:, par, :, D : D + 1])
                ob = outp.tile([128, 2, D], F32, tag="ob", name="ob")
                for j in range(2):
                    nc.vector.tensor_scalar(
                        ob[:, j, :], ps_t[:, par, j, 0:D], rc[:, j : j + 1], None,
                        Alu.mult,
                    )
                nc.sync.dma_start(
                    out_d.rearrange("h (m two p) d -> h m p two d", two=2, p=128)[
                        h, m
                    ],
                    ob,
                )

            # ---- unified emission: interleave projection pieces with the
            # attention tasks they unblock (block m needs chunks <= (m+1)//2),
            # one piece between consecutive tasks so ACT/DVE attention work
            # spreads across the whole run and only the PE paces. Attention
            # tasks stay software-pipelined: QK(i), PV(i-1), tail(i-2).
            i = 0

            credit = [0]

            def pump(n, flush=False):
                # Process tasks in PAIRS with same-kind ops adjacent (2 tails,
                # 2 QKs, 2 PVs): the PE pays a half-speed matmul after each
                # weight-geometry switch (transpose / K=64 QK / K=128 PV), so
                # batching halves the switch count. Credits accumulate across
                # calls so pairs actually form. PV lags QK by 2 tasks so et
                # (exp+mask) is always ready; tails lag by 4.
                nonlocal i
                credit[0] += n
                while credit[0] >= 2 or (flush and credit[0] > 0):
                    k = min(2, credit[0])
                    for d in range(k):
                        if 0 <= i + d - 4 < NTASK:
                            emit_tail(i + d - 4)
                    for d in range(k):
                        if i + d < NTASK:
                            emit_qk(i + d)
                    for d in range(k):
                        if 0 <= i + d - 2 < NTASK:
                            emit_pv(i + d - 2)
                    i += k
                    credit[0] -= k

            # tasks of block m are emitted only after every piece of chunk
            # K(m) = min((m+1)//2, NCH-1): reads must follow their producing
            # writes in program order for Tile to insert the dependency.
            for c in range(NCH):
                pieces = emit_chunk_pieces(c)
                avail = HPC * len(
                    [m for m in range(NBLK) if min((m + 1) // 2, NCH - 1) < c]
                )
                base = i
                for pi, piece in enumerate(pieces):
                    piece()
                    budget = avail - base
                    share = budget * (pi + 1) // len(pieces) - budget * pi // len(
                        pieces
                    )
                    pump(min(share, avail - i - credit[0]))
            pump(NTASK + 4 - i - credit[0], flush=True)
    nc.compile()
    return nc


_CACHE = {}


def _get_nc():
    if "nc" not in _CACHE:
        _CACHE["nc"] = _build_nc()
    return _CACHE["nc"]


def make_in_maps(hidden_states, Wq, bq, Wk, bk, Wv, bv):
    hidden_states = np.asarray(hidden_states, dtype=np.float32)
    Wq = np.asarray(Wq, dtype=np.float32)
    Wk = np.asarray(Wk, dtype=np.float32)
    Wv = np.asarray(Wv, dtype=np.float32)
    bq = np.asarray(bq, dtype=np.float32)
    bk = np.asarray(bk, dtype=np.float32)
    bv = np.asarray(bv, dtype=np.float32)
    scale = 1.0 / float(np.sqrt(D))
    # masks layout [p][kind][128]: masks_np[kind][p][c] -> [p][kind][c]
    masks = np.ascontiguousarray(
        _mask_np().transpose(1, 0, 2).reshape(128, 256)
    ).astype(bfloat16)
    identB = np.eye(128, dtype=np.float32).astype(bfloat16)
    in_maps = []
    for core in range(NCORES):
        b = core // (NCORES // B)
        h0 = HPC * (core % (NCORES // B))
        hsl = [slice(D * (h0 + hh), D * (h0 + hh + 1)) for hh in range(HPC)]
        cols = slice(D * h0, D * (h0 + HPC))
        G = np.empty((HPC, E, 128), np.float32)
        G[0][:, 0:D] = Wk[:, hsl[0]]
        G[0][:, D:128] = Wk[:, hsl[1]]
        G[1][:, 0:D] = Wq[:, hsl[0]] * scale
        G[1][:, D:128] = Wq[:, hsl[1]] * scale
        G[2][:, 0:D] = Wk[:, hsl[2]]
        G[2][:, D:128] = Wq[:, hsl[2]] * scale
        # wqk3[p, g, ki, m] = G[g][ki*128+p, m]
        wqk3 = G.reshape(HPC, KI, 128, 128).transpose(2, 0, 1, 3)
        bqk = np.empty((128, HPC), np.float32)
        bqk[0:D, 0] = bk[hsl[0]]
        bqk[D:128, 0] = bk[hsl[1]]
        bqk[0:D, 1] = bq[hsl[0]] * scale
        bqk[D:128, 1] = bq[hsl[1]] * scale
        bqk[0:D, 2] = bk[hsl[2]]
        bqk[D:128, 2] = bq[hsl[2]] * scale
        # wv[p, ki, n] = Wv[ki*128+p, cols[n]]
        wv_p = Wv[:, cols].reshape(KI, 128, HPC * D).transpose(1, 0, 2)
        bvb = np.broadcast_to(bv[cols], (128, HPC * D)).copy()
        # hT6[p, c, ki, s] = hidden[b].T[ki*128+p, c*512+s]
        hT6 = np.ascontiguousarray(
            hidden_states[b].T.reshape(KI, 128, NCH, PCH).transpose(1, 2, 0, 3)
        ).astype(bfloat16)
        in_maps.append(
            dict(
                hT6=hT6,
                wqk3=np.ascontiguousarray(wqk3).reshape(128, HPC * KI * 128).astype(bfloat16),
                wv=np.ascontiguousarray(wv_p).reshape(128, KI * 192).astype(bfloat16),
                masks=np.ascontiguousarray(masks),
                identB=identB,
                bqk=bqk,
                bvb=bvb,
            )
        )
    return in_maps


def kernel(hidden_states, Wq, bq, Wk, bk, Wv, bv):
    in_maps = make_in_maps(hidden_states, Wq, bq, Wk, bk, Wv, bv)
    res = run_bass_kernel_spmd(_get_nc(), in_maps, list(range(NCORES)))
    kernel.last = res
    out = np.empty((B, S, E), np.float32)
    for core in range(NCORES):
        r = res.results[core]["out"]
        b = core // (NCORES // B)
        h0 = HPC * (core % (NCORES // B))
        for hh in range(HPC):
            out[b, :, D * (h0 + hh) : D * (h0 + hh + 1)] = r[hh]
    return out
